# revision 41
# baseline (speedup 1.0000x reference)
"""Sparse GQA attention (nn_MHA_13950053777893) on 8 TRN2 NeuronCores.

Problem: B=2, Sq=Sk=2048, H=16 q-heads, Hkv=4, D=128, f32.
Reference semantics (prefix-valid key padding mask of length sk per batch):
  - score(t, s) = q.k/sqrt(D) for s <= t + sk - Sq, else exactly -10000
  - softmax over s; for rows t < Sq - sk every score is -10000 -> uniform
    attention = mean over ALL Sk value rows (host fills those rows).
  - exp(-10000 - max) == 0 exactly in f32, so softmax over only the
    causally-allowed band matches the reference's full-row softmax for
    rows with a non-empty band.

Sharding (no collectives, disjoint outputs):
  core c in 0..7: kv group g = c // 2, heads {4g + 2*(c%2), 4g + 2*(c%2) + 1}
  for BOTH batches -> each core does 2 heads x 2 batches = 4 head-instances
  and needs only kv head g. Work is identical across cores.

Device algorithm per head-instance (all matmuls bf16 -> f32 PSUM; output
computed directly in [t, d] layout so no transposes are needed):
  for each 512-wide t-chunk:
    for each 128-row s-block i whose band intersects the chunk:
      tstart = max(t0, 128*floor((s0 + lo)/128))  # band-aligned start
      (fully-masked leading columns are skipped via dskip)
      S^T_psum[s, t] = K^T_i.T @ Q^T[:, tstart:tend]       (PE)
      P^T = exp(S^T / sqrt(D)) -> bf16 SBUF                (ACT)
      diagonal region: P^T = affine_select(P^T, 0)         (GPSIMD)
      for each live 128-wide t-sub-block j:
        po_j[t, 0:129] += P^T-slice.T @ [V_i | 1]          (PE, accumulate)
      (po_j column 128 is the softmax denominator for free)
    per live j: rec = 1/po_j[:,128] (DVE), stn = po_j[:,0:128]*rec (DVE,
    bf16 out); one DMA per chunk into a partition-major DRAM layout
    [B,2,128,SQ//128,D] (1KB contiguous runs/partition; host de-interleaves).
Scalar/ACT is the bottleneck engine (~37us of exp over the banded area at
~1.1ns/col for 512-wide tiles). S^T/exp producers run 2 s-blocks ahead of
the AV consumers (incl. across chunk boundaries) to keep ACT >90% fed;
SBUF pools are sized so no buffer is reused within a build (no WAR sem
traffic on the ACT queue). PE warmup matmuls on a memset tile cover the
DMA prologue so HAM stays unthrottled; tiny critical-path K/Q DMA pieces
are issued first from otherwise-idle engines.
"""

import functools

import numpy as np

B, SQ, SK, H, HKV, D = 2, 2048, 2048, 16, 4, 128
CH = 512  # t-chunk width
N_CORES = 8


@functools.lru_cache(maxsize=4)
def _build(sk_tuple):
    import concourse.bass as bass  # noqa: F401
    import concourse.mybir as mybir
    from concourse.tile import TileContext
    from concourse import bacc

    BF16 = mybir.dt.bfloat16
    F32 = mybir.dt.float32
    sks = list(sk_tuple)

    nc = bacc.Bacc(target_bir_lowering=False, debug=False)
    qt_d = nc.dram_tensor("qt", [B, 2, D, SQ], BF16, kind="ExternalInput")
    kt_d = nc.dram_tensor("kt", [B, D, SK], BF16, kind="ExternalInput")
    vo_d = nc.dram_tensor("vo", [B, 128, SK // 128, D + 1], BF16, kind="ExternalInput")
    # partition-major output layout: per-partition runs are (j x 128) bf16
    # contiguous (up to 1KB) instead of 256B in [t, d]-major -- out-DMA
    # descriptors drain ~3x faster; the host de-interleaves for free.
    out_d = nc.dram_tensor("out", [B, 2, 128, SQ // 128, D], BF16, kind="ExternalOutput")

    scale = float(1.0 / np.sqrt(D))
    NSUB = CH // 128

    with TileContext(nc) as tc:
        with (
            tc.tile_pool(name="big", bufs=1) as big,
            tc.tile_pool(name="pt", bufs=80) as ptp,
            tc.tile_pool(name="rec", bufs=44) as recp,
            tc.tile_pool(name="stn", bufs=24) as stp,
            tc.tile_pool(name="psS", bufs=4, space="PSUM") as psS,
            tc.tile_pool(name="psO", bufs=4, space="PSUM") as psO,
        ):
            # critical first Q piece for (b0,h0) chunk t0=512, issued as the
            # very first gpsimd op so its ~2us DMA latency starts early.
            qt00 = big.tile([D, SQ], BF16, tag="qt00", name="qt00")
            nc.gpsimd.dma_start(out=qt00[:, 512:1024], in_=qt_d[0, 0][:, 512:1024])

            # PE warmup: dependency-free matmuls during the DMA prologue keep
            # HAM from throttling the PE when real matmuls start. The operand
            # is memset on-device (a DMA'd operand would make the warmup WAIT
            # for the DMA queue and delay the first real matmul by ~4us).
            ident = big.tile([128, 128], BF16, tag="ident")
            nc.gpsimd.memset(ident, 1.0)
            pw = psS.tile([128, CH], F32, tag="ps", name="pw")
            for _ in range(18):
                nc.tensor.matmul(pw[:, :128], ident, ident, start=True, stop=True)

            kt = {}
            vo = {}
            for b in range(B):
                sk = sks[b]
                lo = SQ - sk  # first row with a non-empty band
                nsb_total = (sk + 127) // 128
                # only the live ranges are ever read: kt/vo up to the key
                # length, qt from the first banded row. Skipping the dead
                # ranges cuts ~35% of input DMA traffic so the live pieces
                # land sooner.
                klim = 128 * nsb_total
                qlo = 128 * (lo // 128)
                kt[b] = big.tile([D, SK], BF16, tag=f"kt{b}", name=f"kt{b}")
                if b == 0:
                    # the first chunk's first matmul needs only kt[:, :128]
                    # and qt[:, 512:1024]: tiny critical pieces issued from
                    # otherwise-idle engines so compute starts ~3us earlier.
                    nc.scalar.dma_start(out=kt[b][:, :128], in_=kt_d[b][:, :128])
                    kmid = 128 * ((128 + klim) // 256)
                    nc.sync.dma_start(out=kt[b][:, 128:kmid], in_=kt_d[b][:, 128:kmid])
                    nc.sync.dma_start(out=kt[b][:, kmid:klim], in_=kt_d[b][:, kmid:klim])
                else:
                    kmid = 128 * (nsb_total // 2)
                    nc.sync.dma_start(out=kt[b][:, :kmid], in_=kt_d[b][:, :kmid])
                    nc.sync.dma_start(out=kt[b][:, kmid:klim], in_=kt_d[b][:, kmid:klim])
                for hh in range(2):
                    if b == 0 and hh == 0:
                        qt = qt00  # cols 512:1024 already in flight (gpsimd)
                        if qlo < 512:
                            nc.sync.dma_start(out=qt[:, qlo:512], in_=qt_d[b, hh][:, qlo:512])
                        nc.sync.dma_start(out=qt[:, 1024:1536], in_=qt_d[b, hh][:, 1024:1536])
                        nc.sync.dma_start(out=qt[:, 1536:], in_=qt_d[b, hh][:, 1536:])
                    else:
                        qt = big.tile([D, SQ], BF16, tag=f"qt{b}{hh}")
                        qmid = qlo + 128 * ((SQ - qlo) // 256)
                        nc.sync.dma_start(out=qt[:, qlo:qmid], in_=qt_d[b, hh][:, qlo:qmid])
                        nc.sync.dma_start(out=qt[:, qmid:], in_=qt_d[b, hh][:, qmid:])
                    if b not in vo:
                        vo[b] = big.tile(
                            [128, SK // 128, D + 1], BF16, tag=f"vo{b}", name=f"vo{b}"
                        )
                        vmid = (nsb_total + 1) // 2
                        nc.sync.dma_start(
                            out=vo[b][:, :vmid, :], in_=vo_d[b][:, :vmid, :]
                        )
                        nc.sync.dma_start(
                            out=vo[b][:, vmid:nsb_total, :], in_=vo_d[b][:, vmid:nsb_total, :]
                        )
                    oview = out_d[b, hh]  # [128, SQ//128, D] partition-major
                    # chunk descriptors for this head-instance
                    chunks = []
                    for t0 in range(0, SQ, CH):
                        tend = min(t0 + CH, SQ)
                        nsub = (tend - t0) // 128
                        if tend - 1 < lo:
                            continue  # fully uniform rows; host fills
                        sblocks = []
                        for i in range(nsb_total):
                            s0 = 128 * i
                            ts_full = 128 * ((s0 + lo) // 128)
                            if ts_full >= tend:
                                break
                            sblocks.append((i, s0, max(t0, ts_full)))
                        contrib = {}
                        for order, (i, s0, tstart) in enumerate(sblocks):
                            for j in range((tstart - t0) // 128, nsub):
                                contrib.setdefault(j, []).append(order)
                        chunks.append((t0, tend, nsub, sblocks, contrib))

                    flat = [
                        (ci, order)
                        for ci, ch in enumerate(chunks)
                        for order in range(len(ch[3]))
                    ]
                    gidx = {k: n for n, k in enumerate(flat)}
                    pts = {}

                    def emit_producer(ci, order):
                        t0, tend, nsub, sblocks, contrib = chunks[ci]
                        i, s0, tstart = sblocks[order]
                        N = tend - tstart
                        # leading columns with NO valid row (t < s0+lo) are
                        # skipped in MM1/ACT; affine_select writes zeros.
                        dskip = max(0, min(s0 + lo - tstart, N - 1))
                        ps = psS.tile([128, CH], F32, tag="ps", name="ps")
                        nc.tensor.matmul(
                            ps[:, dskip:N],
                            kt[b][:, s0 : s0 + 128],
                            qt[:, tstart + dskip : tend],
                            start=True,
                            stop=True,
                        )
                        pt = ptp.tile([128, CH], BF16, tag="pt", name="pt")
                        pts[(ci, order)] = pt
                        nc.scalar.activation(
                            out=pt[:, dskip:N],
                            in_=ps[:, dskip:N],
                            func=mybir.ActivationFunctionType.Exp,
                            scale=scale,
                        )
                        wm = s0 + lo + 128 - tstart
                        if wm > 0:
                            wm = min(wm, N)
                            # zero entries (tstart+col) - (s0+p) - lo < 0
                            nc.gpsimd.affine_select(
                                out=pt[:, :wm],
                                in_=pt[:, :wm],
                                compare_op=mybir.AluOpType.is_ge,
                                fill=0.0,
                                base=tstart - s0 - lo,
                                channel_multiplier=-1,
                                pattern=[[1, wm]],
                            )

                    # producers run LOOK s-blocks ahead of the AV consumer so
                    # the exp stream never waits on AV completion, including
                    # across chunk boundaries.
                    LOOK = 3
                    cursor = 0
                    for ci, (t0, tend, nsub, sblocks, contrib) in enumerate(chunks):
                        j0 = min(contrib)
                        po = {
                            j: psO.tile([128, 512], F32, tag="po", name=f"po{j}")
                            for j in sorted(contrib)
                        }
                        for order, (i, s0, tstart) in enumerate(sblocks):
                            target = gidx[(ci, order)] + LOOK
                            while cursor <= target and cursor < len(flat):
                                emit_producer(*flat[cursor])
                                cursor += 1
                            for j in range((tstart - t0) // 128, nsub):
                                off = t0 + 128 * j - tstart
                                nc.tensor.matmul(
                                    po[j][:, : D + 1],
                                    pts[(ci, order)][:, off : off + 128],
                                    vo[b][:, i, :],
                                    start=(order == contrib[j][0]),
                                    stop=(order == contrib[j][-1]),
                                )
                        live = sorted(contrib)
                        stn = stp.tile([128, NSUB, 128], BF16, tag="stn")
                        for j in live:
                            rec = recp.tile([128, 1], F32, tag="rec")
                            nc.vector.reciprocal(rec, po[j][:, D : D + 1])
                            nc.vector.tensor_scalar_mul(
                                stn[:, j, :], po[j][:, :D], rec
                            )
                        nc.sync.dma_start(
                            out=oview[:, t0 // 128 + j0 : t0 // 128 + nsub, :],
                            in_=stn[:, j0:nsub, :],
                        )
    nc.finalize()
    return nc


def kernel(q, kv, key_padding_mask):
    import ml_dtypes
    from concourse.bass_utils import run_bass_kernel_spmd

    q = np.asarray(q, dtype=np.float32)
    kv = np.asarray(kv, dtype=np.float32)
    kpm = np.asarray(key_padding_mask)
    sks = tuple(int(x) for x in kpm.sum(axis=1))

    nc = _build(sks)

    bf16 = ml_dtypes.bfloat16
    k_all = kv[:, :, 0]  # (B, SK, HKV, D)
    v_all = kv[:, :, 1]

    in_maps = []
    for c in range(N_CORES):
        g, half = c // 2, c % 2
        heads = [4 * g + 2 * half, 4 * g + 2 * half + 1]
        qt = np.ascontiguousarray(
            q[:, :, heads, :].transpose(0, 2, 3, 1)  # (B, 2, D, SQ)
        ).astype(bf16)
        kt = np.ascontiguousarray(k_all[:, :, g, :].transpose(0, 2, 1)).astype(bf16)
        vo = np.ones((B, SK, D + 1), dtype=np.float32)
        vo[:, :, :D] = v_all[:, :, g, :]
        vo = np.ascontiguousarray(
            vo.reshape(B, SK // 128, 128, D + 1).transpose(0, 2, 1, 3)
        ).astype(bf16)
        in_maps.append({"qt": qt, "kt": kt, "vo": vo})

    import os

    trace = bool(os.environ.get("BASS_MHA_TRACE"))
    if trace:
        try:
            import trace_hook  # noqa: F401  (dev-only NTFF hook shim)
        except ImportError:
            trace = False

    res = run_bass_kernel_spmd(
        nc, in_maps, list(range(N_CORES)),
        trace=trace, trace_cores=[0] if trace else None,
    )
    kernel._last_exec_time_ns = res.exec_time_ns
    kernel._last_trace = res.instructions_and_trace

    out = np.empty((B, SQ, H, D), dtype=np.float32)
    for c in range(N_CORES):
        g, half = c // 2, c % 2
        heads = [4 * g + 2 * half, 4 * g + 2 * half + 1]
        r = np.asarray(res.results[c]["out"], dtype=np.float32)  # (B,2,128,SQ//128,D)
        for b in range(B):
            for hh, h in enumerate(heads):
                out[b, :, h, :] = r[b, hh].transpose(1, 0, 2).reshape(SQ, D)

    # uniform-attention rows: all scores == -10000 -> mean over ALL value rows
    vm = v_all.mean(axis=1)  # (B, HKV, D)
    for b in range(B):
        lo = SQ - sks[b]
        if lo > 0:
            out[b, :lo, :, :] = vm[b, np.arange(H) // (H // HKV), :][None, :, :]
    return out


kernel._last_exec_time_ns = None
kernel._last_trace = None


# revision 42
# speedup vs baseline: 1.0445x; 1.0445x over previous
"""Sparse GQA attention (nn_MHA_13950053777893) on 8 TRN2 NeuronCores.

Problem: B=2, Sq=Sk=2048, H=16 q-heads, Hkv=4, D=128, f32.
Reference semantics (prefix-valid key padding mask of length sk per batch):
  - score(t, s) = q.k/sqrt(D) for s <= t + sk - Sq, else exactly -10000
  - softmax over s; for rows t < Sq - sk every score is -10000 -> uniform
    attention = mean over ALL Sk value rows (host fills those rows).
  - exp(-10000 - max) == 0 exactly in f32, so softmax over only the
    causally-allowed band matches the reference's full-row softmax for
    rows with a non-empty band.

Sharding (no collectives, disjoint outputs):
  core c in 0..7: kv group g = c // 2, heads {4g + 2*(c%2), 4g + 2*(c%2) + 1}
  for BOTH batches -> each core does 2 heads x 2 batches = 4 head-instances
  and needs only kv head g. Work is identical across cores.

Device algorithm per head-instance (all matmuls bf16 -> f32 PSUM; output
computed directly in [t, d] layout so no transposes are needed):
  for each 512-wide t-chunk:
    for each 128-row s-block i whose band intersects the chunk:
      tstart = max(t0, 128*floor((s0 + lo)/128))  # band-aligned start
      (fully-masked leading columns are skipped via dskip)
      S^T_psum[s, t] = K^T_i.T @ Q^T[:, tstart:tend]       (PE)
      P^T = exp(S^T / sqrt(D)) -> bf16 SBUF                (ACT)
      diagonal region: P^T = affine_select(P^T, 0)         (GPSIMD)
      for each live 128-wide t-sub-block j:
        po_j[t, 0:129] += P^T-slice.T @ [V_i | 1]          (PE, accumulate)
      (po_j column 128 is the softmax denominator for free)
    per live j: rec = 1/po_j[:,128] (DVE), stn = po_j[:,0:128]*rec (DVE,
    bf16 out); one DMA per chunk into a partition-major DRAM layout
    [B,2,128,SQ//128,D] (1KB contiguous runs/partition; host de-interleaves).
Scalar/ACT is the bottleneck engine (~37us of exp over the banded area at
~1.1ns/col for 512-wide tiles). S^T/exp producers run 2 s-blocks ahead of
the AV consumers (incl. across chunk boundaries) to keep ACT >90% fed;
SBUF pools are sized so no buffer is reused within a build (no WAR sem
traffic on the ACT queue). PE warmup matmuls on a memset tile cover the
DMA prologue so HAM stays unthrottled; tiny critical-path K/Q DMA pieces
are issued first from otherwise-idle engines.
"""

import functools

import numpy as np

B, SQ, SK, H, HKV, D = 2, 2048, 2048, 16, 4, 128
CH = 512  # t-chunk width
N_CORES = 8


@functools.lru_cache(maxsize=4)
def _build(sk_tuple):
    import concourse.bass as bass  # noqa: F401
    import concourse.mybir as mybir
    from concourse.tile import TileContext
    from concourse import bacc

    BF16 = mybir.dt.bfloat16
    F32 = mybir.dt.float32
    sks = list(sk_tuple)

    nc = bacc.Bacc(target_bir_lowering=False, debug=False)
    qt_d = nc.dram_tensor("qt", [B, 2, D, SQ], BF16, kind="ExternalInput")
    kt_d = nc.dram_tensor("kt", [B, D, SK], BF16, kind="ExternalInput")
    vo_d = nc.dram_tensor("vo", [B, 128, SK // 128, D + 1], BF16, kind="ExternalInput")
    # partition-major output layout: per-partition runs are (j x 128) bf16
    # contiguous (up to 1KB) instead of 256B in [t, d]-major -- out-DMA
    # descriptors drain ~3x faster; the host de-interleaves for free.
    out_d = nc.dram_tensor("out", [B, 2, 128, SQ // 128, D], BF16, kind="ExternalOutput")

    scale = float(1.0 / np.sqrt(D))
    NSUB = CH // 128

    with TileContext(nc) as tc:
        with (
            tc.tile_pool(name="big", bufs=1) as big,
            tc.tile_pool(name="pt", bufs=80) as ptp,
            tc.tile_pool(name="rec", bufs=44) as recp,
            tc.tile_pool(name="stn", bufs=24) as stp,
            tc.tile_pool(name="psS", bufs=4, space="PSUM") as psS,
            tc.tile_pool(name="psO", bufs=4, space="PSUM") as psO,
        ):
            # critical first Q piece for (b0,h0) chunk t0=512, issued as the
            # very first gpsimd op so its ~2us DMA latency starts early.
            qt00 = big.tile([D, SQ], BF16, tag="qt00", name="qt00")
            nc.gpsimd.dma_start(out=qt00[:, 512:1024], in_=qt_d[0, 0][:, 512:1024])

            # PE warmup: dependency-free matmuls during the DMA prologue keep
            # HAM from throttling the PE when real matmuls start. The operand
            # is memset on-device (a DMA'd operand would make the warmup WAIT
            # for the DMA queue and delay the first real matmul by ~4us).
            ident = big.tile([128, 128], BF16, tag="ident")
            nc.gpsimd.memset(ident, 1.0)
            pw = psS.tile([128, CH], F32, tag="ps", name="pw")
            for _ in range(18):
                nc.tensor.matmul(pw[:, :128], ident, ident, start=True, stop=True)

            kt = {}
            vo = {}
            for b in range(B):
                sk = sks[b]
                lo = SQ - sk  # first row with a non-empty band
                nsb_total = (sk + 127) // 128
                # only the live ranges are ever read: kt/vo up to the key
                # length, qt from the first banded row. Skipping the dead
                # ranges cuts ~35% of input DMA traffic so the live pieces
                # land sooner.
                klim = 128 * nsb_total
                qlo = 128 * (lo // 128)
                kt[b] = big.tile([D, SK], BF16, tag=f"kt{b}", name=f"kt{b}")
                if b == 0:
                    # the first chunk's first matmul needs only kt[:, :128]
                    # and qt[:, 512:1024]: tiny critical pieces issued from
                    # otherwise-idle engines so compute starts ~3us earlier.
                    nc.scalar.dma_start(out=kt[b][:, :128], in_=kt_d[b][:, :128])
                    kmid = 128 * ((128 + klim) // 256)
                    nc.sync.dma_start(out=kt[b][:, 128:kmid], in_=kt_d[b][:, 128:kmid])
                    nc.sync.dma_start(out=kt[b][:, kmid:klim], in_=kt_d[b][:, kmid:klim])
                else:
                    kmid = 128 * (nsb_total // 2)
                    nc.sync.dma_start(out=kt[b][:, :kmid], in_=kt_d[b][:, :kmid])
                    nc.sync.dma_start(out=kt[b][:, kmid:klim], in_=kt_d[b][:, kmid:klim])
                for hh in range(2):
                    if b == 0 and hh == 0:
                        qt = qt00  # cols 512:1024 already in flight (gpsimd)
                        if qlo < 512:
                            nc.sync.dma_start(out=qt[:, qlo:512], in_=qt_d[b, hh][:, qlo:512])
                        nc.sync.dma_start(out=qt[:, 1024:1536], in_=qt_d[b, hh][:, 1024:1536])
                        nc.sync.dma_start(out=qt[:, 1536:], in_=qt_d[b, hh][:, 1536:])
                    else:
                        qt = big.tile([D, SQ], BF16, tag=f"qt{b}{hh}")
                        qmid = qlo + 128 * ((SQ - qlo) // 256)
                        nc.sync.dma_start(out=qt[:, qlo:qmid], in_=qt_d[b, hh][:, qlo:qmid])
                        nc.sync.dma_start(out=qt[:, qmid:], in_=qt_d[b, hh][:, qmid:])
                    if b not in vo:
                        vo[b] = big.tile(
                            [128, SK // 128, D + 1], BF16, tag=f"vo{b}", name=f"vo{b}"
                        )
                        vmid = (nsb_total + 1) // 2
                        nc.sync.dma_start(
                            out=vo[b][:, :vmid, :], in_=vo_d[b][:, :vmid, :]
                        )
                        nc.sync.dma_start(
                            out=vo[b][:, vmid:nsb_total, :], in_=vo_d[b][:, vmid:nsb_total, :]
                        )
                    oview = out_d[b, hh]  # [128, SQ//128, D] partition-major
                    # chunk descriptors for this head-instance
                    chunks = []
                    for t0 in range(0, SQ, CH):
                        tend = min(t0 + CH, SQ)
                        nsub = (tend - t0) // 128
                        if tend - 1 < lo:
                            continue  # fully uniform rows; host fills
                        sblocks = []
                        for i in range(nsb_total):
                            s0 = 128 * i
                            ts_full = 128 * ((s0 + lo) // 128)
                            if ts_full >= tend:
                                break
                            sblocks.append((i, s0, max(t0, ts_full)))
                        contrib = {}
                        for order, (i, s0, tstart) in enumerate(sblocks):
                            for j in range((tstart - t0) // 128, nsub):
                                contrib.setdefault(j, []).append(order)
                        chunks.append((t0, tend, nsub, sblocks, contrib))

                    flat = [
                        (ci, order)
                        for ci, ch in enumerate(chunks)
                        for order in range(len(ch[3]))
                    ]
                    gidx = {k: n for n, k in enumerate(flat)}
                    pts = {}

                    def emit_producer(ci, order):
                        t0, tend, nsub, sblocks, contrib = chunks[ci]
                        i, s0, tstart = sblocks[order]
                        N = tend - tstart
                        # leading columns with NO valid row (t < s0+lo) are
                        # skipped in MM1/ACT; affine_select writes zeros.
                        dskip = max(0, min(s0 + lo - tstart, N - 1))
                        ps = psS.tile([128, CH], F32, tag="ps", name="ps")
                        nc.tensor.matmul(
                            ps[:, dskip:N],
                            kt[b][:, s0 : s0 + 128],
                            qt[:, tstart + dskip : tend],
                            start=True,
                            stop=True,
                        )
                        pt = ptp.tile([128, CH], BF16, tag="pt", name="pt")
                        pts[(ci, order)] = pt
                        nc.scalar.activation(
                            out=pt[:, dskip:N],
                            in_=ps[:, dskip:N],
                            func=mybir.ActivationFunctionType.Exp,
                            scale=scale,
                        )
                        wm = s0 + lo + 128 - tstart
                        if wm > 0:
                            wm = min(wm, N)
                            # zero entries (tstart+col) - (s0+p) - lo < 0
                            nc.gpsimd.affine_select(
                                out=pt[:, :wm],
                                in_=pt[:, :wm],
                                compare_op=mybir.AluOpType.is_ge,
                                fill=0.0,
                                base=tstart - s0 - lo,
                                channel_multiplier=-1,
                                pattern=[[1, wm]],
                            )

                    # producers run LOOK s-blocks ahead of the AV consumer so
                    # the exp stream never waits on AV completion, including
                    # across chunk boundaries.
                    LOOK = 2
                    cursor = 0
                    for ci, (t0, tend, nsub, sblocks, contrib) in enumerate(chunks):
                        j0 = min(contrib)
                        po = {
                            j: psO.tile([128, 512], F32, tag="po", name=f"po{j}")
                            for j in sorted(contrib)
                        }
                        for order, (i, s0, tstart) in enumerate(sblocks):
                            target = gidx[(ci, order)] + LOOK
                            while cursor <= target and cursor < len(flat):
                                emit_producer(*flat[cursor])
                                cursor += 1
                            for j in range((tstart - t0) // 128, nsub):
                                off = t0 + 128 * j - tstart
                                nc.tensor.matmul(
                                    po[j][:, : D + 1],
                                    pts[(ci, order)][:, off : off + 128],
                                    vo[b][:, i, :],
                                    start=(order == contrib[j][0]),
                                    stop=(order == contrib[j][-1]),
                                )
                        live = sorted(contrib)
                        stn = stp.tile([128, NSUB, 128], BF16, tag="stn")
                        for j in live:
                            rec = recp.tile([128, 1], F32, tag="rec")
                            nc.vector.reciprocal(rec, po[j][:, D : D + 1])
                            nc.vector.tensor_scalar_mul(
                                stn[:, j, :], po[j][:, :D], rec
                            )
                        nc.sync.dma_start(
                            out=oview[:, t0 // 128 + j0 : t0 // 128 + nsub, :],
                            in_=stn[:, j0:nsub, :],
                        )
    nc.finalize()
    return nc


def kernel(q, kv, key_padding_mask):
    import ml_dtypes
    from concourse.bass_utils import run_bass_kernel_spmd

    q = np.asarray(q, dtype=np.float32)
    kv = np.asarray(kv, dtype=np.float32)
    kpm = np.asarray(key_padding_mask)
    sks = tuple(int(x) for x in kpm.sum(axis=1))

    nc = _build(sks)

    bf16 = ml_dtypes.bfloat16
    k_all = kv[:, :, 0]  # (B, SK, HKV, D)
    v_all = kv[:, :, 1]

    in_maps = []
    for c in range(N_CORES):
        g, half = c // 2, c % 2
        heads = [4 * g + 2 * half, 4 * g + 2 * half + 1]
        qt = np.ascontiguousarray(
            q[:, :, heads, :].transpose(0, 2, 3, 1)  # (B, 2, D, SQ)
        ).astype(bf16)
        kt = np.ascontiguousarray(k_all[:, :, g, :].transpose(0, 2, 1)).astype(bf16)
        vo = np.ones((B, SK, D + 1), dtype=np.float32)
        vo[:, :, :D] = v_all[:, :, g, :]
        vo = np.ascontiguousarray(
            vo.reshape(B, SK // 128, 128, D + 1).transpose(0, 2, 1, 3)
        ).astype(bf16)
        in_maps.append({"qt": qt, "kt": kt, "vo": vo})

    import os

    trace = bool(os.environ.get("BASS_MHA_TRACE"))
    if trace:
        try:
            import trace_hook  # noqa: F401  (dev-only NTFF hook shim)
        except ImportError:
            trace = False

    res = run_bass_kernel_spmd(
        nc, in_maps, list(range(N_CORES)),
        trace=trace, trace_cores=[0] if trace else None,
    )
    kernel._last_exec_time_ns = res.exec_time_ns
    kernel._last_trace = res.instructions_and_trace

    out = np.empty((B, SQ, H, D), dtype=np.float32)
    for c in range(N_CORES):
        g, half = c // 2, c % 2
        heads = [4 * g + 2 * half, 4 * g + 2 * half + 1]
        r = np.asarray(res.results[c]["out"], dtype=np.float32)  # (B,2,128,SQ//128,D)
        for b in range(B):
            for hh, h in enumerate(heads):
                out[b, :, h, :] = r[b, hh].transpose(1, 0, 2).reshape(SQ, D)

    # uniform-attention rows: all scores == -10000 -> mean over ALL value rows
    vm = v_all.mean(axis=1)  # (B, HKV, D)
    for b in range(B):
        lo = SQ - sks[b]
        if lo > 0:
            out[b, :lo, :, :] = vm[b, np.arange(H) // (H // HKV), :][None, :, :]
    return out


kernel._last_exec_time_ns = None
kernel._last_trace = None
